# revision 13
# baseline (speedup 1.0000x reference)
"""Trainium2 Bass kernel for nn_MixedMlp (soft-mixture MoE MLP).

Math (per batch row b):
    cn = LayerNorm(c); x = [z, cn]
    coeff = softmax(gateMLP(x))                       # [E]
    l0 = elu(sum_e coeff_e (x @ w0_e + b0_e))
    l1 = elu(sum_e coeff_e ([z, l0] @ w1_e + b1_e))
    out = sum_e coeff_e ([z, l1] @ w2_e + b2_e)

Kernel strategy (8 cores, data-parallel over B=8192):
  * Activations kept feature-major ([features, batch]) so every layer is a
    single PSUM-accumulated GEMM with contraction over K = E*in using
    coeff-scaled inputs:  out^T = sum_e W_e^T (coeff_e ⊙ X^T).
  * ELU computed as s = elu(x)+1 = relu(x) + min(exp(x), 1); the -1 is folded
    into the next layer's bias host-side (b' = b - sum_k w[k]).
  * Layer 2 (out dim 16) uses stacked per-expert outputs [(e,o), b] = W2stk^T X
    mixed by an expanded-coeff elementwise multiply and a selector matmul that
    directly yields row-major [b, 16] output.
  * coeff broadcasts ([128, B] tiles of per-expert gate values) are built by
    DMA replication from a small DRAM staging buffer - no engine time.
  * Matmuls run as float32r (full PE rate at free-dim >= 256, ~fp32 storage).
"""

import numpy as np
from contextlib import ExitStack

import concourse.bass as bass
import concourse.bacc as bacc
import concourse.tile as tile
import concourse.mybir as mybir
from concourse import bass_utils
from concourse.bass import AP

F32 = mybir.dt.float32
F32R = mybir.dt.float32r
AF = mybir.ActivationFunctionType
OP = mybir.AluOpType

N_CORES = 8
B = 8192
R = B // N_CORES          # rows per core = 1024
LATENT, CIN, HID, ACTD, E, GH = 32, 128, 256, 16, 8, 128
IN0, INTER = LATENT + CIN, HID + LATENT
LN_EPS = 1e-5
BT = 512                  # batch tile (matmul moving free dim)
NBT = R // BT             # 2
NCH = R // 128            # 8 b-chunks per core

_CACHE = {}


def _build_program():
    nc = bacc.Bacc("TRN2", target_bir_lowering=False, debug=False,
                   num_devices=N_CORES)

    dram = {}
    def din(name, shape, dt=F32):
        dram[name] = nc.dram_tensor(name, list(shape), dt, kind="ExternalInput").ap()
        return dram[name]

    zr_d = din("zrep", (128, R), F32R)   # host-prepared z^T replicated 4x
    c_d = din("c", (R, CIN))
    w0z_d = din("w0z", (128, 512), F32R)    # [128, kt*128 + m] kt=2, M=256
    w0c_d = din("w0c", (128, 2048), F32R)   # kt=8
    w1z_d = din("w1z", (128, 512), F32R)
    w1h_d = din("w1h", (128, 4096), F32R)   # kt=16
    w2s_d = din("w2s", (128, 384), F32R)    # [W2a(32 rows); W2b; W2c]
    s2_d = din("s2sel", (128, 16), F32R)
    i128_d = din("i128", (128, 128))
    g0z_d = din("g0z", (32, 128), F32R)
    g0c_d = din("g0c", (128, 128), F32R)
    g1w_d = din("g1w", (128, 128), F32R)
    g2w_d = din("g2w", (128, 8), F32R)
    b01_d = din("b01", (8, 512), F32R)      # [b0 | b1']
    ck_d = din("consts128", (128, 6))    # cols: lngam lnbet eps g0b g1b b2col
    g2b_d = din("g2b", (8, 1))
    on8_d = din("ones8", (8, 1), F32R)
    onr_d = din("onesr8", (1, 8), F32R)
    out_d = nc.dram_tensor("out", [R, ACTD], F32, kind="ExternalOutput").ap()

    with tile.TileContext(nc) as tc, ExitStack() as ctx:
        wp = ctx.enter_context(tc.tile_pool(name="wp", bufs=1))       # weights
        big = ctx.enter_context(tc.tile_pool(name="big", bufs=1))     # persistent activations
        sp = ctx.enter_context(tc.tile_pool(name="sp", bufs=4))       # small temps
        er = ctx.enter_context(tc.tile_pool(name="er", bufs=4))       # elu temps [128,512]
        sc = ctx.enter_context(tc.tile_pool(name="sc", bufs=6))       # scaled-input tiles
        pt = ctx.enter_context(tc.tile_pool(name="pt", bufs=2, space="PSUM"))   # transposes
        pm = ctx.enter_context(tc.tile_pool(name="pm", bufs=4, space="PSUM"))   # big matmuls
        psm = ctx.enter_context(tc.tile_pool(name="psm", bufs=1, space="PSUM")) # small matmuls (shared tag)
        po = ctx.enter_context(tc.tile_pool(name="po", bufs=1, space="PSUM"))   # out matmuls
        dstage = ctx.enter_context(tc.tile_pool(name="dstage", bufs=1, space="DRAM"))

        # ---------------- weight / const loads ----------------
        def wload(dr, eng=None):
            t = wp.tile(list(dr.shape), dr.dtype, name=f"w_{dr.tensor.name}")
            (eng or nc.sync).dma_start(t[:], dr[:])
            return t

        # critical-path-first on the sync queue; big expert weights on scalar
        ck = wload(ck_d)
        i128 = wload(i128_d)
        g0z = wload(g0z_d); g0c = wload(g0c_d); g1w = wload(g1w_d); g2w = wload(g2w_d)
        g2b = wload(g2b_d); on8 = wload(on8_d); onr = wload(onr_d)
        b01 = wload(b01_d)
        w0z = wload(w0z_d, nc.scalar); w0c = wload(w0c_d, nc.scalar)
        w1z = wload(w1z_d, nc.scalar); w1h = wload(w1h_d, nc.scalar)
        w2s = wload(w2s_d, nc.scalar); s2 = wload(s2_d, nc.scalar)
        lng, lnb, epsc = ck[:, 0:1], ck[:, 1:2], ck[:, 2:3]
        g0b, g1b, b2c = ck[:, 3:4], ck[:, 4:5], ck[:, 5:6]

        # ---------------- persistent activation tiles ----------------
        cnT = big.tile([128, R], F32R)     # LayerNormed c, feature-major
        zrep = big.tile([128, R], F32R)    # z^T replicated 4x along partitions
        h0 = big.tile([128, R], F32R)      # gate hidden 1 (= elu+1)
        h1 = big.tile([128, R], F32R)
        eL = big.tile([8, R], F32R)        # exp(gate logits)
        coeffN = big.tile([8, R], F32R)    # softmax coeffs
        s0a = big.tile([128, R], F32R)     # layer0 out (= elu+1), feat 0..127
        s0b = big.tile([128, R], F32R)     # feat 128..255
        s1a = big.tile([128, R], F32R)
        s1b = big.tile([128, R], F32R)
        zs = [big.tile([128, R], F32R, name=f"zs{q}") for q in range(2)]
        cb = [big.tile([128, R], F32R, name=f"cb{e}") for e in range(E)]
        cbe16 = big.tile([128, R], F32R)

        # ---------------- stage A: LayerNorm(c) + transposes ----------------
        ctall = big.tile([128, NCH * CIN], F32)   # c as [b%128, (chunk, feat)]
        nc.sync.dma_start(
            ctall[:], AP(c_d.tensor, 0, [[CIN, 128], [128 * CIN, NCH], [1, CIN]]))
        nc.sync.dma_start(zrep[:], zr_d[:])
        for j in range(NCH):
            js = slice(128 * j, 128 * (j + 1))
            ct = ctall[:, 128 * j:128 * (j + 1)]
            stats = sp.tile([128, 6], F32, tag="st")
            nc.vector.bn_stats(stats[:], ct[:])
            mv = sp.tile([128, 2], F32, tag="mv")
            nc.vector.bn_aggr(mv[:], stats[:])
            lnv = sp.tile([128, 1], F32, tag="sd")
            nc.scalar.activation(lnv[:], mv[:, 1:2], AF.Ln, bias=epsc[:])
            rstd = sp.tile([128, 1], F32, tag="rs")
            nc.scalar.activation(rstd[:], lnv[:], AF.Exp, scale=-0.5)
            y = sp.tile([128, 128], F32, tag="y")
            nc.vector.tensor_scalar(y[:], ct[:], mv[:, 0:1], rstd[:],
                                    OP.subtract, OP.mult)
            yT = pt.tile([128, 128], F32, tag="tp")
            nc.tensor.transpose(yT[:], y[:], i128[:])
            # cn = y^T * gamma + beta   (per-partition scalars, PSUM->SBUF)
            nc.vector.tensor_scalar(cnT[:, js], yT[:], lng[:], lnb[:],
                                    OP.mult, OP.add)


        # ---------------- stage B: gate ----------------
        for bt in range(NBT):
            bs = slice(BT * bt, BT * (bt + 1))
            pre0 = pm.tile([128, BT], F32, tag="mm")
            nc.tensor.matmul(pre0[:], g0z[:], zrep[0:32, bs],
                             start=True, stop=False)
            nc.tensor.matmul(pre0[:], g0c[:], cnT[:, bs],
                             start=False, stop=True)
            e0 = er.tile([128, BT], F32, tag="e")
            nc.scalar.activation(e0[:], pre0[:], AF.Exp, bias=g0b[:])
            r0 = er.tile([128, BT], F32, tag="r")
            nc.vector.tensor_scalar(r0[:], pre0[:], g0b[:], 0.0, OP.add, OP.max)
            nc.vector.scalar_tensor_tensor(h0[:, bs], e0[:], 1.0, r0[:],
                                           OP.min, OP.add)

            pre1 = pm.tile([128, BT], F32, tag="mm")
            nc.tensor.matmul(pre1[:], g1w[:], h0[:, bs], start=True, stop=True)
            e1 = er.tile([128, BT], F32, tag="e")
            nc.scalar.activation(e1[:], pre1[:], AF.Exp, bias=g1b[:])
            r1 = er.tile([128, BT], F32, tag="r")
            nc.vector.tensor_scalar(r1[:], pre1[:], g1b[:], 0.0, OP.add, OP.max)
            nc.vector.scalar_tensor_tensor(h1[:, bs], e1[:], 1.0, r1[:],
                                           OP.min, OP.add)

            pre2 = psm.tile([8, BT], F32, tag="sm")
            nc.tensor.matmul(pre2[:], g2w[:], h1[:, bs], start=True, stop=True)
            nc.scalar.activation(eL[:, bs], pre2[:], AF.Exp, bias=g2b[:])
            sume = psm.tile([1, BT], F32, tag="sm")
            nc.tensor.matmul(sume[:], on8[:], eL[:, bs], start=True, stop=True)
            rsum = sp.tile([1, BT], F32, tag="rsm")
            nc.vector.reciprocal(rsum[:], sume[:])
            rsr = sp.tile([1, BT], F32R, tag="rsr")
            nc.vector.tensor_copy(rsr[:], rsum[:])
            rbc = psm.tile([8, BT], F32, tag="sm")
            nc.tensor.matmul(rbc[:], onr[:], rsr[:], start=True, stop=True)
            nc.vector.tensor_mul(coeffN[:, bs], eL[:, bs], rbc[:])

        # ---------------- coeff staging + broadcast DMAs (per b-tile) ----------------
        cbz = [big.tile([128, R], F32R, name=f"cbz{q}") for q in range(2)]
        cstage = dstage.tile([8, R], F32R)
        ctens = cstage.tensor
        for bt in range(NBT):
            bs = slice(BT * bt, BT * (bt + 1))
            o = BT * bt
            nc.sync.dma_start(cstage[:, bs], coeffN[:, bs])
            q2 = [nc.sync, nc.scalar]
            for e in range(E):
                q2[e % 2].dma_start(cb[e][:, bs],
                                    AP(ctens, e * R + o, [[0, 128], [1, BT]]))
            for q in range(2):
                q2[q].dma_start(cbz[q][:, bs],
                                AP(ctens, 4 * q * R + o, [[R, 4], [0, 32], [1, BT]]))
            nc.scalar.dma_start(cbe16[:, bs],
                                AP(ctens, o, [[R, 8], [0, 16], [1, BT]]))
            # scaled z quads (reused by L0 and L1)
            for q in range(2):
                nc.vector.tensor_mul(zs[q][:, bs], zrep[:, bs], cbz[q][:, bs])

        # ---------------- layer helper ----------------
        def elu_plus1(ps, dst, bs):
            ee = er.tile([128, BT], F32, tag="e")
            nc.scalar.activation(ee[:], ps[:], AF.Exp)
            rr = er.tile([128, BT], F32, tag="r")
            nc.vector.tensor_scalar_max(rr[:], ps[:], 0.0)
            nc.vector.scalar_tensor_tensor(dst[:, bs], ee[:], 1.0, rr[:],
                                           OP.min, OP.add)

        # ---------------- layers 0 and 1 ----------------
        # K order: [z-quad0(128) z-quad1(128)] + [h per expert, h = cn (L0) or
        # s0 halves (L1)].  kt loop is outermost so each scaled tile is
        # consumed by both M-tiles immediately (short lifetime).
        def expert_layer(bt, wz, wh, bias_off, srcs, tag, dsts):
            bs = slice(BT * bt, BT * (bt + 1))
            ps = [pm.tile([128, BT], F32, tag="mm", name=f"ps{tag}{bt}_{mt}")
                  for mt in range(2)]
            for mt in range(2):
                nc.tensor.matmul(ps[mt][:],
                                 b01[:, bias_off + 128 * mt:bias_off + 128 * (mt + 1)],
                                 coeffN[:, bs], start=True, stop=False)
            for kt in range(2):
                for mt in range(2):
                    nc.tensor.matmul(ps[mt][:], wz[:, 256 * kt + 128 * mt:
                                                       256 * kt + 128 * (mt + 1)],
                                     zs[kt][:, bs], start=False, stop=False)
            nkt = len(srcs)
            for kt in range(nkt):
                e, srct = srcs[kt]
                t = sc.tile([128, BT], F32R, tag=tag, name=f"x{tag}{bt}_{kt}")
                eng = nc.gpsimd if kt % 4 == 3 else nc.vector
                eng.tensor_mul(t[:], srct[:, bs], cb[e][:, bs])
                for mt in range(2):
                    nc.tensor.matmul(ps[mt][:], wh[:, 256 * kt + 128 * mt:
                                                       256 * kt + 128 * (mt + 1)],
                                     t[:, :], start=False,
                                     stop=(kt == nkt - 1))
            for mt in range(2):
                elu_plus1(ps[mt], dsts[mt], bs)

        for bt in range(NBT):
            expert_layer(bt, w0z, w0c, 0,
                         [(e, cnT) for e in range(E)], "sc0", (s0a, s0b))
        for bt in range(NBT):
            expert_layer(bt, w1z, w1h, 256,
                         [(e, t) for e in range(E) for t in (s0a, s0b)],
                         "sc1", (s1a, s1b))

        # ---------------- layer 2 ----------------
        for bt in range(NBT):
            bs = slice(BT * bt, BT * (bt + 1))
            per2 = pm.tile([128, BT], F32, tag="mm")
            nc.tensor.matmul(per2[:], w2s[0:32, 0:128], zrep[0:32, bs],
                             start=True, stop=False)
            nc.tensor.matmul(per2[:], w2s[:, 128:256], s1a[:, bs],
                             start=False, stop=False)
            nc.tensor.matmul(per2[:], w2s[:, 256:384], s1b[:, bs],
                             start=False, stop=True)
            mixed = er.tile([128, BT], F32R, tag="mx")
            nc.vector.scalar_tensor_tensor(mixed[:], per2[:], b2c[:], cbe16[:, bs],
                                           OP.add, OP.mult)
            otb = sp.tile([128, (BT // 128) * ACTD], F32, tag="ot")
            for jj in range(BT // 128):
                op = po.tile([128, ACTD], F32, tag="op")
                nc.tensor.matmul(op[:], mixed[:, 128 * jj:128 * (jj + 1)],
                                 s2[:], start=True, stop=True)
                nc.vector.tensor_copy(otb[:, ACTD * jj:ACTD * (jj + 1)], op[:])
            nc.sync.dma_start(
                AP(out_d.tensor, BT * bt * ACTD,
                   [[ACTD, 128], [128 * ACTD, BT // 128], [1, ACTD]]),
                otb[:])

    nc.compile()
    return nc


def _host_prep(inputs):
    f = lambda a: np.ascontiguousarray(np.asarray(a, dtype=np.float32))
    w0, b0 = f(inputs["w0"]), f(inputs["b0"])
    w1, b1 = f(inputs["w1"]), f(inputs["b1"])
    w2, b2 = f(inputs["w2"]), f(inputs["b2"])
    g0w, g0b = f(inputs["g0w"]), f(inputs["g0b"])
    g1w, g1b = f(inputs["g1w"]), f(inputs["g1b"])
    g2w, g2b = f(inputs["g2w"]), f(inputs["g2b"])
    ln_g, ln_b = f(inputs["ln_g"]), f(inputs["ln_b"])

    def ksb(wstk, nkt, m):   # [nkt*128, m] -> [128, nkt*m]
        return np.ascontiguousarray(
            wstk.reshape(nkt, 128, m).transpose(1, 0, 2).reshape(128, nkt * m))

    w0z = ksb(w0[:, :LATENT, :].reshape(E * LATENT, HID), 2, HID)
    w0c = ksb(w0[:, LATENT:, :].reshape(E * CIN, HID), 8, HID)
    w1z = ksb(w1[:, :LATENT, :].reshape(E * LATENT, HID), 2, HID)
    w1h = ksb(w1[:, LATENT:, :].reshape(E * HID, HID), 16, HID)
    b1f = b1 - w1[:, LATENT:, :].sum(axis=1)

    w2stk = w2.transpose(1, 0, 2).reshape(INTER, E * ACTD)   # [288, 128]
    w2s = np.zeros((128, 384), np.float32)
    w2s[:32, 0:128] = w2stk[0:32]
    w2s[:, 128:256] = w2stk[32:160]
    w2s[:, 256:384] = w2stk[160:288]
    b2f = b2 - w2[:, LATENT:, :].sum(axis=1)                 # [8,16]

    d = {
        "w0z": w0z, "w0c": w0c, "w1z": w1z, "w1h": w1h, "w2s": w2s,
        "s2sel": np.ascontiguousarray(np.tile(np.eye(ACTD, dtype=np.float32), (E, 1))),
        "i128": np.eye(128, dtype=np.float32),
        "g0z": np.ascontiguousarray(g0w[:LATENT]),
        "g0c": np.ascontiguousarray(g0w[LATENT:]),
        "g1w": g1w, "g2w": g2w,
        "b01": np.ascontiguousarray(np.concatenate([b0, b1f], axis=1)),
        "consts128": np.stack([
            ln_g, ln_b, np.full(128, LN_EPS, np.float32),
            g0b, g1b - g1w.sum(0), b2f.reshape(128)], axis=1),
        "g2b": (g2b - g2w.sum(0)).reshape(8, 1),
        "ones8": np.ones((8, 1), np.float32),
        "onesr8": np.ones((1, 8), np.float32),
    }
    return {k: np.ascontiguousarray(v, dtype=np.float32) for k, v in d.items()}


def make_in_maps(inputs):
    wmap = _host_prep(inputs)
    z = np.ascontiguousarray(np.asarray(inputs["z"], dtype=np.float32))
    c = np.ascontiguousarray(np.asarray(inputs["c"], dtype=np.float32))
    in_maps = []
    for i in range(N_CORES):
        m = dict(wmap)
        zsh = z[i * R:(i + 1) * R]
        m["zrep"] = np.ascontiguousarray(np.tile(zsh.T, (4, 1)))
        m["c"] = np.ascontiguousarray(c[i * R:(i + 1) * R])
        in_maps.append(m)
    return in_maps


def kernel(**inputs):
    if "nc" not in _CACHE:
        _CACHE["nc"] = _build_program()
    nc = _CACHE["nc"]
    in_maps = make_in_maps(inputs)
    res = bass_utils.run_bass_kernel_spmd(nc, in_maps, core_ids=list(range(N_CORES)))
    return np.concatenate([res.results[i]["out"] for i in range(N_CORES)], axis=0)
